# revision 1
# baseline (speedup 1.0000x reference)
"""Trainium2 Bass kernel for nn_CELossTotalEval (CE-shift + unlikelihood + 2x CE).

Data-parallel over the batch dim: 16 batch rows -> 8 cores x 2 rows.
Each core streams its (512, 16384) f32 shards of out0/out1/out2 once from HBM
(HWDGE, ~2 MB tiles) and produces tiny per-row statistics:
  - chunked row sums (4 chunks of 4096) for each tensor            [ACT]
  - out0 row max + first-occurrence argmax via two stages:         [DVE]
      stage 1: 128-wide sub-chunk maxes while streaming,
      stage 2: re-gather only the winning 128-wide slice (indirect DMA)
        and resolve the position inside it.
  - gathered target probabilities via indirect DMA                  [SWDGE]
The host combines the per-core statistics into the scalar loss (log/div on
16x256-sized arrays); all data-touching math stays on device.
"""

import sys

sys.path.insert(0, "/opt/trn_rl_repo")

import numpy as np

import concourse.bass as bass
import concourse.mybir as mybir
import concourse.tile as tile

N, T, V = 16, 256, 16384
NCORES = 8
NB = N // NCORES          # batch rows per core
ROWS = NB * T             # 512 flattened (n, t) rows per core
P = 128                   # SBUF partitions
R = ROWS // P             # 4 row-tiles per core
FD = 4096                 # streamed v-chunk width
NJ = V // FD              # 4 v-chunks per row
SUB = 128                 # argmax sub-chunk width
NSUB = V // SUB           # 128 sub-chunks per row
NGRAM = 4
UL_MIN = np.float32(1e-20)
IGNORE = -1

F32 = mybir.dt.float32
I32 = mybir.dt.int32


def _split_multiwaits(nc, max_waits=1):
    """Hoist extra semaphore waits into standalone single-wait EventSemaphore
    instructions on the same engine.

    The walrus build in this container rejects instructions carrying more than
    one sync wait ("Too many sync wait commands"), but Tile emits multi-wait
    sync_info.  A preceding single-wait EventSemaphore on the same engine is
    semantically identical (the sequencer stalls until each wait passes).
    """
    for fn in nc.m.functions:
        for blk in fn.blocks:
            out = []
            changed = False
            for ins in blk.instructions:
                si = ins.sync_info
                waits = list(si.on_wait) if si and si.on_wait else []
                if len(waits) > max_waits:
                    changed = True
                    for k, w in enumerate(waits[: len(waits) - max_waits]):
                        out.append(
                            mybir.InstEventSemaphore(
                                name=f"{ins.name}-hw{k}",
                                opcode="EventSemaphore",
                                engine=ins.engine,
                                ins=[],
                                outs=[],
                                sync_info=mybir.SyncInfo(
                                    on_wait=[w], on_update=[]
                                ),
                            )
                        )
                    si.on_wait = waits[len(waits) - max_waits:]
                out.append(ins)
            if changed:
                blk.instructions = out
    return nc


def build_bass(split_waits=True, reps=1):
    nc = bass.Bass()

    xs = [
        nc.dram_tensor(f"x{i}", [ROWS, V], F32, kind="ExternalInput")
        for i in range(3)
    ]
    offs_in = [
        nc.dram_tensor(f"off{i}", [P, R], I32, kind="ExternalInput")
        for i in range(3)
    ]
    rs_out = [
        nc.dram_tensor(f"rs{i}", [P, R * NJ], F32, kind="ExternalOutput")
        for i in range(3)
    ]
    rm_out = nc.dram_tensor("rm0", [P, R], F32, kind="ExternalOutput")
    crev_out = nc.dram_tensor("crev0", [P, R], F32, kind="ExternalOutput")
    wrev_out = nc.dram_tensor("wrev0", [P, R], F32, kind="ExternalOutput")
    pt_out = [
        nc.dram_tensor(f"pt{i}", [P, R], F32, kind="ExternalOutput")
        for i in range(3)
    ]

    with tile.TileContext(nc) as tc:
        with (
            tc.tile_pool(name="singles", bufs=1) as singles,
            tc.tile_pool(name="stream0", bufs=3) as stream0,
            tc.tile_pool(name="stream1", bufs=2) as stream1,
            tc.tile_pool(name="stream2", bufs=2) as stream2,
            tc.tile_pool(name="scratch", bufs=2) as scratch,
            tc.tile_pool(name="argmax", bufs=2) as amx,
        ):
            stream_pools = [stream0, stream1, stream2]

            # (127 - k) ramp, one row of SUB entries per partition.
            rev128 = singles.tile([P, SUB], F32)
            nc.gpsimd.iota(
                rev128[:],
                pattern=[[-1, SUB]],
                base=SUB - 1,
                channel_multiplier=0,
                allow_small_or_imprecise_dtypes=True,
            )
            # Per-partition row-base element offsets for each row-tile:
            # base[p] = (r*128 + p) * V  (exact in f32: < 2^24 after scaling).
            rowbase = singles.tile([P, R], F32)
            for r in range(R):
                nc.gpsimd.iota(
                    rowbase[:, r:r + 1],
                    pattern=[[0, 1]],
                    base=r * P * V,
                    channel_multiplier=V,
                    allow_small_or_imprecise_dtypes=True,
                )

            # Gather offsets (element indices into the flat (ROWS*V) shard).
            offs_t = []
            for i in range(3):
                ot = singles.tile([P, R], I32)
                nc.gpsimd.dma_start(out=ot[:], in_=offs_in[i][:, :])
                offs_t.append(ot)

            # Target-probability gathers: HW indirect DMA takes ONE offset per
            # partition (gathering out-free-size contiguous elements), so one
            # gather per row-tile column.
            pt_t = []
            for i in range(3):
                pt = singles.tile([P, R], F32)
                for r in range(R):
                    nc.gpsimd.indirect_dma_start(
                        out=pt[:, r:r + 1],
                        out_offset=None,
                        in_=xs[i][:, :],
                        in_offset=bass.IndirectOffsetOnAxis(
                            ap=offs_t[i][:, r:r + 1], axis=1
                        ),
                    )
                pt_t.append(pt)

            # Persistent per-row statistic accumulators.
            rs_t = [
                singles.tile([P, R * NJ], F32, name=f"rs_t{i}") for i in range(3)
            ]
            rm_t = singles.tile([P, R], F32)
            crev_t = singles.tile([P, R], F32)
            wrev_t = singles.tile([P, R], F32)

            for _rep in range(reps):
                for r in range(R):
                    # Sub-chunk maxes for this row-tile accumulate here.
                    cmax = amx.tile([P, NSUB], F32, tag="cmax")
                    for j in range(NJ):
                        slot = r * NJ + j
                        for i in range(3):
                            tl = stream_pools[i].tile([P, FD], F32, tag=f"s{i}")
                            # Split stream loads across both HWDGE rings
                            # (SP and ACT) to keep more SDMA slots in flight.
                            dma_eng = (
                                nc.sync if (i == 0 or (i == 2 and j % 2 == 0))
                                else nc.scalar
                            )
                            dma_eng.dma_start(
                                out=tl[:],
                                in_=xs[i][r * P:(r + 1) * P,
                                          j * FD:(j + 1) * FD],
                            )
                            # Row-sum partial for this v-chunk (ACT engine).
                            sc = scratch.tile([P, FD], F32, tag="act")
                            nc.scalar.activation(
                                out=sc[:],
                                in_=tl[:],
                                func=mybir.ActivationFunctionType.Copy,
                                accum_out=rs_t[i][:, slot:slot + 1],
                            )
                            if i == 0:
                                # 128-wide sub-chunk maxes (FD/SUB per call).
                                nc.vector.reduce_max(
                                    out=cmax[:, j * (FD // SUB):(j + 1) * (FD // SUB)],
                                    in_=tl[:].rearrange(
                                        "p (c w) -> p c w", w=SUB
                                    ),
                                    axis=mybir.AxisListType.X,
                                )

                    # Row max over the NSUB sub-chunk maxes.
                    nc.vector.reduce_max(
                        out=rm_t[:, r:r + 1],
                        in_=cmax[:],
                        axis=mybir.AxisListType.X,
                    )
                    # First sub-chunk attaining the row max, as 127-c.
                    eqc = amx.tile([P, NSUB], F32, tag="eqc")
                    nc.vector.tensor_scalar(
                        out=eqc[:],
                        in0=cmax[:],
                        scalar1=rm_t[:, r:r + 1],
                        scalar2=None,
                        op0=mybir.AluOpType.is_ge,
                    )
                    nc.vector.tensor_tensor(
                        out=eqc[:], in0=eqc[:], in1=rev128[:],
                        op=mybir.AluOpType.mult,
                    )
                    nc.vector.reduce_max(
                        out=crev_t[:, r:r + 1], in_=eqc[:],
                        axis=mybir.AxisListType.X,
                    )
                    # Element offset of the winning sub-chunk:
                    #   base + (127 - crev) * 128
                    #   = rowbase[r] + 127*128 - crev*128.
                    goff_f = amx.tile([P, 1], F32, tag="goff_f")
                    nc.vector.tensor_scalar(
                        out=goff_f[:], in0=crev_t[:, r:r + 1],
                        scalar1=-float(SUB), scalar2=float((SUB - 1) * SUB),
                        op0=mybir.AluOpType.mult,
                        op1=mybir.AluOpType.add,
                    )
                    nc.vector.tensor_tensor(
                        out=goff_f[:], in0=goff_f[:], in1=rowbase[:, r:r + 1],
                        op=mybir.AluOpType.add,
                    )
                    goff_i = amx.tile([P, 1], I32, tag="goff_i")
                    nc.vector.tensor_copy(out=goff_i[:], in_=goff_f[:])
                    # Re-gather the winning 128-wide slice from HBM.
                    gth = amx.tile([P, SUB], F32, tag="gth")
                    nc.gpsimd.indirect_dma_start(
                        out=gth[:],
                        out_offset=None,
                        in_=xs[0][:, :],
                        in_offset=bass.IndirectOffsetOnAxis(
                            ap=goff_i[:], axis=1
                        ),
                    )
                    # First position inside the slice attaining the max.
                    eqw = amx.tile([P, SUB], F32, tag="eqw")
                    nc.vector.tensor_scalar(
                        out=eqw[:], in0=gth[:],
                        scalar1=rm_t[:, r:r + 1], scalar2=None,
                        op0=mybir.AluOpType.is_ge,
                    )
                    nc.vector.tensor_tensor(
                        out=eqw[:], in0=eqw[:], in1=rev128[:],
                        op=mybir.AluOpType.mult,
                    )
                    nc.vector.reduce_max(
                        out=wrev_t[:, r:r + 1], in_=eqw[:],
                        axis=mybir.AxisListType.X,
                    )

            # Ship the tiny statistics out.
            for i in range(3):
                nc.gpsimd.dma_start(out=rs_out[i][:, :], in_=rs_t[i][:])
                nc.gpsimd.dma_start(out=pt_out[i][:, :], in_=pt_t[i][:])
            nc.gpsimd.dma_start(out=rm_out[:, :], in_=rm_t[:])
            nc.gpsimd.dma_start(out=crev_out[:, :], in_=crev_t[:])
            nc.gpsimd.dma_start(out=wrev_out[:, :], in_=wrev_t[:])

    return _split_multiwaits(nc) if split_waits else nc


def make_offsets(tgt0, tgt1):
    """Per-core (P, R) int32 element offsets into the flat (ROWS*V) shards.

    SBUF partition p of row-tile r holds flat row fl = r*128 + p, which is
    (n_loc, t) = divmod(fl, T).  out0 gathers tgt0[n, t+1] (CE shift); out1 and
    out2 gather tgt1[n, t].  Rows with no target (t == T-1 for out0) point at
    element 0 of the row and are ignored on the host.
    """
    offs = [np.zeros((NCORES, P, R), np.int32) for _ in range(3)]
    fl = np.arange(ROWS)
    n_loc, t = divmod(fl, T)
    base = fl * V
    for c in range(NCORES):
        t0c = np.asarray(tgt0[c * NB:(c + 1) * NB]).astype(np.int64)
        t1c = np.asarray(tgt1[c * NB:(c + 1) * NB]).astype(np.int64)
        g0 = np.where(t < T - 1, np.clip(t0c[n_loc, np.minimum(t + 1, T - 1)], 0, None), 0)
        g1 = np.clip(t1c[n_loc, t], 0, None)
        offs[0][c] = (base + g0).reshape(R, P).T
        offs[1][c] = (base + g1).reshape(R, P).T
        offs[2][c] = (base + g1).reshape(R, P).T
    return offs


def combine(per_core, tgt0, tgt1):
    """Host-side reconstruction of the loss from per-core statistics."""
    rowsum = np.zeros((3, N, T), np.float64)
    ptgt = np.zeros((3, N, T), np.float64)
    rowmax = np.zeros((N, T), np.float64)
    pred = np.zeros((N, T), np.int64)

    for c in range(NCORES):
        res = per_core[c]
        nsl = slice(c * NB, (c + 1) * NB)
        for i in range(3):
            # rs[p, r*NJ + j]  ->  rowsum[fl = r*128+p] = sum_j
            rs = np.asarray(res[f"rs{i}"], np.float64).reshape(P, R, NJ)
            rowsum[i, nsl] = rs.sum(axis=2).T.reshape(NB, T)
            pt = np.asarray(res[f"pt{i}"], np.float64)  # (P, R)
            ptgt[i, nsl] = pt.T.reshape(NB, T)
        rm = np.asarray(res["rm0"], np.float64)         # (P, R)
        crev = np.asarray(res["crev0"], np.float64)
        wrev = np.asarray(res["wrev0"], np.float64)
        rowmax[nsl] = rm.T.reshape(NB, T)
        c_idx = (SUB - 1) - crev
        w_idx = (SUB - 1) - wrev
        pred[nsl] = (c_idx * SUB + w_idx).astype(np.int64).T.reshape(NB, T)

    tgt0 = np.asarray(tgt0).astype(np.int64)
    tgt1 = np.asarray(tgt1).astype(np.int64)

    def ce(i, tgt, tslice):
        valid = tgt != IGNORE
        nll = np.log(rowsum[i][:, tslice]) - np.log(ptgt[i][:, tslice])
        return np.where(valid, nll, 0.0).sum() / valid.sum()

    ce0 = ce(0, tgt0[:, 1:], slice(0, T - 1))
    ce1 = ce(1, tgt1, slice(None))
    ce2 = ce(2, tgt1, slice(None))

    # Unlikelihood on out0: 4-gram repeat mask over the argmax tokens.
    J = T - NGRAM
    ngrams = np.stack([pred[:, k:k + J] for k in range(NGRAM)], axis=-1)
    eq = (ngrams[:, :, None, :] == ngrams[:, None, :, :]).all(-1)
    earlier = np.tril(np.ones((J, J), bool), k=-1)
    rep = (eq & earlier).any(-1)
    mask = np.zeros((N, T), bool)
    for k in range(NGRAM):
        mask[:, k:k + J] |= rep
    g = rowmax.astype(np.float32)
    one_minus = np.maximum(np.float32(1.0) - np.exp(g), UL_MIN)
    ul = (-np.log(one_minus.astype(np.float64)) * mask).sum()

    return np.asarray(ce0 + ul + ce1 + ce2, dtype=np.float32)


_NC_CACHE = None


def kernel(out0, out1, out2, tgt0, tgt1):
    global _NC_CACHE
    from concourse.bass_utils import run_bass_kernel_spmd

    if _NC_CACHE is None:
        _NC_CACHE = build_bass()
    nc = _NC_CACHE

    out0 = np.asarray(out0, np.float32)
    out1 = np.asarray(out1, np.float32)
    out2 = np.asarray(out2, np.float32)
    offs = make_offsets(tgt0, tgt1)

    in_maps = []
    for c in range(NCORES):
        nsl = slice(c * NB, (c + 1) * NB)
        in_maps.append({
            "x0": np.ascontiguousarray(out0[nsl].reshape(ROWS, V)),
            "x1": np.ascontiguousarray(out1[nsl].reshape(ROWS, V)),
            "x2": np.ascontiguousarray(out2[nsl].reshape(ROWS, V)),
            "off0": np.ascontiguousarray(offs[0][c]),
            "off1": np.ascontiguousarray(offs[1][c]),
            "off2": np.ascontiguousarray(offs[2][c]),
        })

    def run_once():
        return run_bass_kernel_spmd(nc, in_maps, list(range(NCORES))).results

    def spot_check(results):
        """Cheap host-side consistency check (one row per tensor per core)
        to catch rare transient device corruption; O(N*V) host work total."""
        for c in range(NCORES):
            r0 = results[c]
            for i, nm in enumerate(["x0", "x1", "x2"]):
                x = in_maps[c][nm]
                p, r = (37 * (c + i)) % P, (c + i) % R
                fl = r * P + p
                exp = x[fl].sum(dtype=np.float64)
                got = np.asarray(r0[f"rs{i}"], np.float64).reshape(P, R, NJ)[p, r].sum()
                if abs(got - exp) > 1e-3 * abs(exp):
                    return False
                off = int(offs[i][c][p, r])
                if np.asarray(r0[f"pt{i}"])[p, r] != x.reshape(-1)[off]:
                    return False
                if i == 0 and np.asarray(r0["rm0"])[p, r] != x[fl].max():
                    return False
        return True

    results = run_once()
    if not spot_check(results):
        results = run_once()
    return combine(results, tgt0, tgt1)

